# revision 10
# baseline (speedup 1.0000x reference)
"""Trainium2 Bass kernel for nn_Attn_17738214933129.

Dense transformer attention block:
  Q/K/V projections from n_loc=2048 -> feat=512 (8 heads x 64),
  structural-bias softmax added to scaled QK^T scores, softmax, PV,
  output projection back to n_loc=2048.

Sharding: data-parallel over batch (16 -> 2 per core) across 8 NeuronCores,
weights replicated, no collectives.

v3 design (per core, rows R = 2*512 = 1024):
  - Q/K projections in fp8e4 with DoubleRow perf mode (2 k-tiles per
    matmul, 2x PE throughput).  Weights pre-scaled by 64 on host so fp8
    stays in normal range; the 64Q*64K scale plus the reference's 1/64
    score scale is folded into the attention exp (scale = 2^-18).
  - V projection in bf16 directly into token-major layout V[r, f],
    augmented with a ones column per head so PV emits softmax row-sums.
  - Scores computed TRANSPOSED: ST[k, q] = KT^T @ QT per (b, h) - no P
    transposes.  attn_mask is folded into str_mat on host (-1e4 fill).
  - The structural softmax sm^T (scaled by 2^18, normalization and scale
    folded into a diag-scaled PE transpose) is ADDED into the score PSUM
    with an identity matmul; one merged [128,1024] exp per (kt, head-pair)
    then yields E = exp(S + sm) directly.
  - Softmax normalization after PV: reciprocal_approx_fast of the rowsum
    row, partition-broadcast via a K=1 PE matmul, multiplied on DVE into
    the out-proj lhsT layout xT.  Odd heads shifted to partitions 64-127
    with a SBUF->SBUF DMA on the gpsimd ring.
  - Output projection bf16; out DMA'd bf16, upcast on host.  Out-proj of
    batch 0 is interleaved into batch 1's attention; batch 1's psum->sbuf
    copies run on ACT to unload DVE.
  - All input DMAs on the SP ring in exact consumption order.
"""

import sys

import numpy as np

try:
    import concourse.bass as bass  # noqa: F401
except Exception:  # pragma: no cover - path fallback
    sys.path.insert(0, "/opt/trn_rl_repo")

import ml_dtypes

import concourse.bacc as bacc
import concourse.tile as tile
from concourse import mybir
from concourse.bass_utils import run_bass_kernel_spmd

BF16 = mybir.dt.bfloat16
F32 = mybir.dt.float32
FP8 = mybir.dt.float8e4
AF = mybir.ActivationFunctionType
ALU = mybir.AluOpType
DR = mybir.MatmulPerfMode.DoubleRow

B, S, NLOC = 16, 512, 2048
FEAT, H, DH = 512, 8, 64
NCORES = 8
BL = B // NCORES          # batch per core = 2
R = BL * S                # rows per core = 1024
KT_N = NLOC // 128        # 16 contraction tiles for projections
FT_N = FEAT // 128        # 4 feature tiles (= head pairs)
QT_N = S // 128           # 4 query tiles per batch element
NL_N = NLOC // 512        # 4 output column chunks
EXP_SCALE = float(2.0 ** -18)

_CACHE = {}


def _build(use_bias):
    nc = bacc.Bacc(
        "TRN2",
        target_bir_lowering=False,
        debug=False,
        enable_asserts=False,
        num_devices=NCORES,
    )

    # q/k pre-transposed+tiled fp8: [p, i, r] = x[r, i*128+p].
    d_q8 = nc.dram_tensor("q8", [128, KT_N, R], FP8, kind="ExternalInput").ap()
    d_k8 = nc.dram_tensor("k8", [128, KT_N, R], FP8, kind="ExternalInput").ap()
    d_v = nc.dram_tensor("v16", [128, KT_N, R], BF16, kind="ExternalInput").ap()
    # masked str pre-tiled bf16: [p, b, qt*S+c] (mask folded in as -1e4).
    d_str = nc.dram_tensor("strm", [128, BL, QT_N * S], BF16, kind="ExternalInput").ap()
    # weights pre-tiled: [p, i, f] = W.T[i*128+p, f] (wq/wk scaled by 64, fp8).
    d_wq8 = nc.dram_tensor("wq8", [128, KT_N, FEAT], FP8, kind="ExternalInput").ap()
    d_wk8 = nc.dram_tensor("wk8", [128, KT_N, FEAT], FP8, kind="ExternalInput").ap()
    d_wv = nc.dram_tensor("wv16", [128, KT_N, FEAT], BF16, kind="ExternalInput").ap()
    d_wo = nc.dram_tensor("wo16", [128, FT_N, NLOC], BF16, kind="ExternalInput").ap()
    d_id = nc.dram_tensor("ident", [128, 128], BF16, kind="ExternalInput").ap()
    # biases (only used when nonzero): bq/bk pre-scaled by 64, feat-major.
    d_bq = nc.dram_tensor("bq2", [128, FT_N], F32, kind="ExternalInput").ap()
    d_bk = nc.dram_tensor("bk2", [128, FT_N], F32, kind="ExternalInput").ap()
    d_bv = nc.dram_tensor("bv2", [1, FEAT], BF16, kind="ExternalInput").ap()
    d_bo = nc.dram_tensor("bo2", [1, NLOC], BF16, kind="ExternalInput").ap()
    d_out = nc.dram_tensor("out", [R, NLOC], BF16, kind="ExternalOutput").ap()

    with tile.TileContext(nc) as tc:
        with (
            tc.tile_pool(name="consts", bufs=1) as cpool,
            tc.tile_pool(name="weights", bufs=1) as wpool,
            tc.tile_pool(name="persist", bufs=1) as ppool,
            tc.tile_pool(name="qkin", bufs=6) as qkpool,
            tc.tile_pool(name="vin", bufs=1) as vpool,
            tc.tile_pool(name="smwork", bufs=1) as mpool,
            tc.tile_pool(name="diagp", bufs=4) as dgpool,
            tc.tile_pool(name="attn", bufs=4) as apool,
            tc.tile_pool(name="recs", bufs=2) as recpool,
            tc.tile_pool(name="rts", bufs=4) as rtpool,
            tc.tile_pool(name="xoddp", bufs=2) as xopool,
            tc.tile_pool(name="ostage", bufs=2) as opool,
            tc.tile_pool(name="psumA", bufs=2, space="PSUM") as psA,
            tc.tile_pool(name="psumB", bufs=4, space="PSUM") as psB,
        ):
            # ---- constants ----------------------------------------------
            ident = cpool.tile([128, 128], BF16, tag="ident", name="ident")
            ones = cpool.tile([128, 128], BF16, tag="ones", name="ones")
            nc.vector.memset(ones[:], 1.0)
            nc.scalar.dma_start(ident[:], d_id[:])

            # str (mask pre-folded) on the ACT hwdge ring.
            strb = [None, None]
            for b in range(BL):
                strb[b] = mpool.tile([128, QT_N, S], BF16, tag=f"str{b}", name=f"str{b}")
                nc.scalar.dma_start(
                    strb[b][:].rearrange("p a c -> p (a c)"), d_str[:, b, :]
                )

            biases = {}
            if use_bias:
                bq_t = cpool.tile([128, FT_N], F32, tag="bq", name="bq")
                nc.scalar.dma_start(bq_t[:], d_bq[:])
                bk_t = cpool.tile([128, FT_N], F32, tag="bk", name="bk")
                nc.scalar.dma_start(bk_t[:], d_bk[:])
                bv_t = cpool.tile([1, FEAT], BF16, tag="bv", name="bv")
                nc.scalar.dma_start(bv_t[:], d_bv[:])
                bo_t = cpool.tile([1, NLOC], BF16, tag="bo", name="bo")
                nc.scalar.dma_start(bo_t[:], d_bo[:])
                biases = {"bq": bq_t, "bk": bk_t, "bv": bv_t, "bo": bo_t}

            # ---- structural softmax prep (ACT/DVE, overlaps Q proj) ------
            # u[b] = exp(masked str) with fused rowsums; diag = ident*ru
            u_t = [None, None]
            diag = [[None] * QT_N, [None] * QT_N]
            for b in range(BL):
                u_t[b] = mpool.tile([128, QT_N, S], BF16, tag=f"u{b}", name=f"u{b}")
                usum = mpool.tile([128, QT_N], F32, tag=f"us{b}", name=f"us{b}")
                for qt in range(QT_N):
                    nc.scalar.activation(
                        u_t[b][:, qt, :], strb[b][:, qt, :], AF.Exp,
                        accum_out=usum[:, qt : qt + 1],
                    )
                ru = mpool.tile([128, QT_N], F32, tag=f"ru{b}", name=f"ru{b}")
                nc.vector.reciprocal_approx_fast(ru[:], usum[:])
                for qt in range(QT_N):
                    dg = dgpool.tile([128, 128], BF16, tag=f"dg{b}{qt}", name=f"dg{b}{qt}")
                    nc.vector.tensor_scalar(
                        dg[:], ident[:], ru[:, qt : qt + 1], None, op0=ALU.mult
                    )
                    diag[b][qt] = dg

            # ---- persistent activations ---------------------------------
            QT = [ppool.tile([128, R], BF16, tag=f"QT{i}", name=f"QT{i}") for i in range(FT_N)]
            KT = [ppool.tile([128, R], BF16, tag=f"KT{i}", name=f"KT{i}") for i in range(FT_N)]
            V2 = [ppool.tile([128, H, DH + 1], BF16, tag=f"V2{i}", name=f"V2{i}") for i in range(R // 128)]
            # smTs[b][kt]: 2^18 * sm^T, bf16
            smTs = [[None] * QT_N, [None] * QT_N]
            xT = [
                [ppool.tile([128, S], BF16, tag=f"xT{b}{j}", name=f"xT{b}{j}") for j in range(FT_N)]
                for b in range(BL)
            ]

            def grab_half(idx):
                # 8 concurrent [128,512] projection groups: 2x2 halves from
                # psA ([128,1024] tiles) + 4 from psB.
                if idx < 4:
                    if idx % 2 == 0:
                        t = psA.tile([128, 1024], F32, tag="ps", name="ps")
                        grab_half._last = t
                    t = grab_half._last
                    return t[:, (idx % 2) * 512 : (idx % 2 + 1) * 512]
                return psB.tile([128, 512], F32, tag="ps", name="ps")[:]

            def proj_fp8(dst, d_x, d_w, w_tag, bias_nm):
                w_t = wpool.tile([128, KT_N, FEAT], FP8, tag=w_tag, name=w_tag)
                groups = {}
                for ft in range(FT_N):
                    for rc in range(2):
                        groups[(ft, rc)] = grab_half(ft * 2 + rc)
                for i in range(KT_N // 2):
                    if i % 2 == 0:
                        j = i // 2
                        nc.sync.dma_start(
                            w_t[:, 4 * j : 4 * j + 4, :], d_w[:, 4 * j : 4 * j + 4, :]
                        )
                    xc = qkpool.tile([128, 2, R], FP8, tag="xin", name="xin")
                    nc.sync.dma_start(xc[:], d_x[:, 2 * i : 2 * i + 2, :])
                    for ft in range(FT_N):
                        for rc in range(2):
                            nc.tensor.matmul(
                                groups[(ft, rc)],
                                lhsT=w_t[:, 2 * i : 2 * i + 2, ft * 128 : (ft + 1) * 128],
                                rhs=xc[:, :, rc * 512 : (rc + 1) * 512],
                                start=(i == 0),
                                stop=(i == KT_N // 2 - 1),
                                perf_mode=DR,
                            )
                for ft in range(FT_N):
                    for rc in range(2):
                        if use_bias:
                            nc.vector.tensor_scalar(
                                dst[ft][:, rc * 512 : (rc + 1) * 512],
                                groups[(ft, rc)],
                                biases[bias_nm][:, ft : ft + 1],
                                None,
                                op0=ALU.add,
                            )
                        else:
                            nc.vector.tensor_copy(
                                dst[ft][:, rc * 512 : (rc + 1) * 512],
                                groups[(ft, rc)],
                            )

            def smt_phase(b):
                # sm^T via diag-scaled PE transpose, exp on ACT -> esmT bf16
                for kt in range(QT_N):
                    smtp = psB.tile([128, 512], F32, tag="ps", name="smtp")
                    for qt in range(QT_N):
                        nc.tensor.matmul(
                            smtp[:, qt * 128 : (qt + 1) * 128],
                            lhsT=u_t[b][:, qt, kt * 128 : (kt + 1) * 128],
                            rhs=diag[b][qt][:],
                            start=(qt == 0),
                            stop=(qt == QT_N - 1),
                        )
                    est = ppool.tile([128, S], BF16, tag=f"esm{b}{kt}", name=f"esm{b}{kt}")
                    nc.scalar.activation(est[:], smtp[:], AF.Exp)
                    smTs[b][kt] = est

            proj_fp8(QT, d_q8, d_wq8, "wq8", "bq")
            smt_phase(0)
            proj_fp8(KT, d_k8, d_wk8, "wk8", "bk")
            smt_phase(1)

            # ---- V projection (bf16, token-major, ones-augmented) --------
            # Group-major (per row-tile) over fully-staged v halves, using
            # only psB banks so batch-0 attention can interleave on psA.
            wv = wpool.tile([128, KT_N, FEAT], BF16, tag="wv", name="wv")
            nc.sync.dma_start(wv[:], d_wv[:])
            vst = []
            for half in range(2):
                vh = vpool.tile([128, KT_N, 512], BF16, tag=f"vst{half}", name=f"vst{half}")
                nc.sync.dma_start(vh[:], d_v[:, :, half * 512 : (half + 1) * 512])
                vst.append(vh)
            # wo after all other inputs on the SP ring.
            wo = wpool.tile([128, FT_N, NLOC], BF16, tag="wo", name="wo")
            nc.sync.dma_start(wo[:], d_wo[:])

            def v_group(rt):
                vg = psB.tile([128, 512], F32, tag="ps", name="vg")
                if use_bias:
                    nc.tensor.matmul(
                        vg[:], lhsT=ones[0:1, :], rhs=biases["bv"][0:1, :],
                        start=True, stop=False,
                    )
                vh = vst[rt // 4]
                for i in range(KT_N):
                    nc.tensor.matmul(
                        vg[:],
                        lhsT=vh[:, i, (rt % 4) * 128 : (rt % 4 + 1) * 128],
                        rhs=wv[:, i, :],
                        start=(i == 0 and not use_bias),
                        stop=(i == KT_N - 1),
                    )
                nc.vector.memset(V2[rt][:], 1.0)
                nc.vector.tensor_copy(
                    V2[rt][:, :, 0:DH],
                    vg[:].rearrange("p (h d) -> p h d", h=H),
                )

            # ---- attention (transposed scores) ---------------------------
            ets = {}

            def scores_part(b, hp):
                for kt in range(QT_N):
                    sp = psA.tile([128, 1024], F32, tag="ps", name="ps")
                    for hs in range(2):
                        hb = hs * 64
                        nc.tensor.matmul(
                            sp[:, hs * 512 : (hs + 1) * 512],
                            lhsT=KT[hp][hb : hb + 64, b * S + kt * 128 : b * S + (kt + 1) * 128],
                            rhs=QT[hp][hb : hb + 64, b * S : (b + 1) * S],
                            start=True,
                            stop=True,
                        )
                    e0 = apool.tile([128, 1024], BF16, tag="e0", name="e0")
                    nc.scalar.activation(e0[:], sp[:], AF.Exp, scale=EXP_SCALE)
                    et = apool.tile([128, 1024], BF16, tag="et", name="et")
                    nc.vector.tensor_tensor(
                        et[:].rearrange("p (a c) -> p a c", a=2),
                        e0[:].rearrange("p (a c) -> p a c", a=2),
                        smTs[b][kt][:].unsqueeze(1).broadcast_to((128, 2, 512)),
                        op=ALU.mult,
                    )
                    ets[(b, hp, kt)] = et

            def pv_part(b, hp):
                yps = [psB.tile([128, 512], F32, tag="ps", name="yps") for _ in range(2)]
                for kt in range(QT_N):
                    for hs in range(2):
                        h = 2 * hp + hs
                        nc.tensor.matmul(
                            yps[hs][0:65, :],
                            lhsT=V2[b * QT_N + kt][:, h, :],
                            rhs=ets[(b, hp, kt)][:, hs * 512 : (hs + 1) * 512],
                            start=(kt == 0),
                            stop=(kt == QT_N - 1),
                        )
                    ets[(b, hp, kt)] = None
                # normalization: copy raw rowsums to SBUF, broadcast with a
                # K=1 PE matmul, one wide approx-reciprocal, multiply to xT.
                recp = recpool.tile([65, 1024], BF16, tag="rec", name="recp")
                nc.vector.tensor_copy(recp[64:65, 0:512], yps[0][64:65, :])
                nc.vector.tensor_copy(recp[64:65, 512:1024], yps[1][64:65, :])
                rtp = psA.tile([128, 1024], F32, tag="ps", name="rtp")
                for hs in range(2):
                    nc.tensor.matmul(
                        rtp[0:64, hs * 512 : (hs + 1) * 512],
                        lhsT=ones[64:65, 0:64],
                        rhs=recp[64:65, hs * 512 : (hs + 1) * 512],
                        start=True,
                        stop=True,
                    )
                rsb = rtpool.tile([64, 1024], F32, tag="rt", name="rt")
                nc.vector.reciprocal_approx_fast(rsb[:], rtp[0:64, :])
                nc.vector.tensor_tensor(
                    xT[b][hp][0:64, :], yps[0][0:64, :], rsb[:, 0:512], op=ALU.mult
                )
                xod = xopool.tile([64, 512], BF16, tag="xod", name="xod")
                nc.vector.tensor_tensor(
                    xod[:], yps[1][0:64, :], rsb[:, 512:1024], op=ALU.mult
                )
                nc.gpsimd.dma_start(xT[b][hp][64:128, :], xod[:])

            def outproj_qt(b, qt, copy_engine):
                row0 = b * S + qt * 128
                ot = opool.tile([128, NLOC], BF16, tag="ot", name="ot")
                for nlc in range(NL_N):
                    ops = psB.tile([128, 512], F32, tag="ps", name="ops")
                    if use_bias:
                        nc.tensor.matmul(
                            ops[:],
                            lhsT=ones[0:1, :],
                            rhs=biases["bo"][0:1, nlc * 512 : (nlc + 1) * 512],
                            start=True,
                            stop=False,
                        )
                    for ft in range(FT_N):
                        nc.tensor.matmul(
                            ops[:],
                            lhsT=xT[b][ft][:, qt * 128 : (qt + 1) * 128],
                            rhs=wo[:, ft, nlc * 512 : (nlc + 1) * 512],
                            start=(ft == 0 and not use_bias),
                            stop=(ft == FT_N - 1),
                        )
                    if copy_engine == "act":
                        nc.scalar.activation(
                            ot[:, nlc * 512 : (nlc + 1) * 512], ops[:], AF.Copy
                        )
                    else:
                        nc.vector.tensor_copy(
                            ot[:, nlc * 512 : (nlc + 1) * 512], ops[:]
                        )
                    if nlc % 2 == 1:
                        ring = nc.gpsimd if (b == 1 and nlc == 1) else nc.sync
                        ring.dma_start(
                            d_out[row0 : row0 + 128, (nlc - 1) * 512 : (nlc + 1) * 512],
                            ot[:, (nlc - 1) * 512 : (nlc + 1) * 512],
                        )

            # batch-0 attention interleaved into the V projection groups
            for rt in range(4):
                v_group(rt)
            scores_part(0, 0)
            v_group(4)
            scores_part(0, 1)
            pv_part(0, 0)
            v_group(5)
            scores_part(0, 2)
            pv_part(0, 1)
            v_group(6)
            scores_part(0, 3)
            pv_part(0, 2)
            v_group(7)
            pv_part(0, 3)
            # batch-1 attention with batch-0 out-proj interleaved
            for hp in range(FT_N):
                scores_part(1, hp)
                pv_part(1, hp)
                outproj_qt(0, hp, "dve")
            for qt in range(QT_N):
                outproj_qt(1, qt, "act")

    nc.compile()
    return nc


def _prep_inputs(q, k, v, str_mat, attn_mask, Wq, bq, Wk, bk, Wv, bv, Wo, bo):
    bf = ml_dtypes.bfloat16
    f8 = ml_dtypes.float8_e4m3

    def pretile_T(x, dt):
        # [R, NLOC] -> [128, KT_N, R] with [p, i, r] = x[r, i*128+p]
        return np.ascontiguousarray(
            x.reshape(R, KT_N, 128).transpose(2, 1, 0).astype(dt)
        )

    def pretile_w(wT, n, dt):
        # [n*128, width] -> [128, n, width]
        return np.ascontiguousarray(
            wT.reshape(n, 128, wT.shape[1]).transpose(1, 0, 2).astype(dt)
        )

    wq8 = pretile_w((64.0 * Wq).T.astype(np.float32), KT_N, f8)
    wk8 = pretile_w((64.0 * Wk).T.astype(np.float32), KT_N, f8)
    wv16 = pretile_w(np.ascontiguousarray(Wv.T), KT_N, bf)
    wo16 = pretile_w(np.ascontiguousarray(Wo.T), FT_N, bf)

    ident = np.eye(128, dtype=bf)
    bq2 = np.ascontiguousarray((64.0 * bq).astype(np.float32).reshape(FT_N, 128).T)
    bk2 = np.ascontiguousarray((64.0 * bk).astype(np.float32).reshape(FT_N, 128).T)
    bv2 = bv[None, :].astype(bf)
    bo2 = bo[None, :].astype(bf)

    # fold attn_mask into str: masked entries -> -1e4 (exp -> 0 in f32)
    strf = np.where(
        np.asarray(attn_mask) == 0, np.float32(-1e4),
        np.asarray(str_mat, dtype=np.float32),
    )

    in_maps = []
    for c in range(NCORES):
        sl = slice(c * BL, (c + 1) * BL)
        strt = np.ascontiguousarray(
            strf[sl].reshape(BL * QT_N, 128, S).transpose(1, 0, 2)
            .reshape(128, BL, QT_N * S).astype(bf)
        )
        in_maps.append(
            {
                "q8": pretile_T(np.asarray(q)[sl].reshape(R, NLOC), f8),
                "k8": pretile_T(np.asarray(k)[sl].reshape(R, NLOC), f8),
                "v16": pretile_T(np.asarray(v)[sl].reshape(R, NLOC), bf),
                "strm": strt,
                "wq8": wq8,
                "wk8": wk8,
                "wv16": wv16,
                "wo16": wo16,
                "ident": ident,
                "bq2": bq2,
                "bk2": bk2,
                "bv2": bv2,
                "bo2": bo2,
            }
        )
    return in_maps


def kernel(q, k, v, str_mat, attn_mask, Wq, bq, Wk, bk, Wv, bv, Wo, bo):
    use_bias = bool(
        np.any(np.asarray(bq))
        or np.any(np.asarray(bk))
        or np.any(np.asarray(bv))
        or np.any(np.asarray(bo))
    )
    key = ("nc", use_bias)
    if key not in _CACHE:
        _CACHE[key] = _build(use_bias)
    nc = _CACHE[key]
    in_maps = _prep_inputs(
        q, k, v, str_mat, attn_mask, Wq, bq, Wk, bk, Wv, bv, Wo, bo
    )
    res = run_bass_kernel_spmd(nc, in_maps, core_ids=list(range(NCORES)))
    out = np.empty((B, S, NLOC), dtype=np.float32)
    for c in range(NCORES):
        out[c * BL : (c + 1) * BL] = (
            res.results[c]["out"].astype(np.float32).reshape(BL, S, NLOC)
        )
    return out


# revision 21
# speedup vs baseline: 1.0846x; 1.0846x over previous
"""Trainium2 Bass kernel for nn_Attn_17738214933129.

Dense transformer attention block:
  Q/K/V projections from n_loc=2048 -> feat=512 (8 heads x 64),
  structural-bias softmax added to scaled QK^T scores, softmax, PV,
  output projection back to n_loc=2048.

Sharding: data-parallel over batch (16 -> 2 per core) across 8 NeuronCores,
weights replicated, no collectives.

v3 design (per core, rows R = 2*512 = 1024):
  - Q/K projections in fp8e4 with DoubleRow perf mode (2 k-tiles per
    matmul, 2x PE throughput).  Weights pre-scaled by 64 on host so fp8
    stays in normal range; the 64Q*64K scale plus the reference's 1/64
    score scale is folded into the attention exp (scale = 2^-18).
  - V projection in bf16 directly into token-major layout V[r, f],
    augmented with a ones column per head so PV emits softmax row-sums.
  - Scores computed TRANSPOSED: ST[k, q] = KT^T @ QT per (b, h) - no P
    transposes.  attn_mask is folded into str_mat on host (-1e4 fill).
  - The structural softmax sm^T (scaled by 2^18, normalization and scale
    folded into a diag-scaled PE transpose) is ADDED into the score PSUM
    with an identity matmul; one merged [128,1024] exp per (kt, head-pair)
    then yields E = exp(S + sm) directly.
  - Softmax normalization after PV: reciprocal_approx_fast of the rowsum
    row, partition-broadcast via a K=1 PE matmul, multiplied on DVE into
    the out-proj lhsT layout xT.  Odd heads shifted to partitions 64-127
    with a SBUF->SBUF DMA on the gpsimd ring.
  - Output projection bf16; out DMA'd bf16, upcast on host.  Out-proj of
    batch 0 is interleaved into batch 1's attention; batch 1's psum->sbuf
    copies run on ACT to unload DVE.
  - All input DMAs on the SP ring in exact consumption order.
"""

import sys

import numpy as np

try:
    import concourse.bass as bass  # noqa: F401
except Exception:  # pragma: no cover - path fallback
    sys.path.insert(0, "/opt/trn_rl_repo")

import ml_dtypes

import concourse.bacc as bacc
import concourse.tile as tile
from concourse import mybir
from concourse.bass_utils import run_bass_kernel_spmd

BF16 = mybir.dt.bfloat16
F32 = mybir.dt.float32
FP8 = mybir.dt.float8e4
AF = mybir.ActivationFunctionType
ALU = mybir.AluOpType
DR = mybir.MatmulPerfMode.DoubleRow

B, S, NLOC = 16, 512, 2048
FEAT, H, DH = 512, 8, 64
NCORES = 8
BL = B // NCORES          # batch per core = 2
R = BL * S                # rows per core = 1024
KT_N = NLOC // 128        # 16 contraction tiles for projections
FT_N = FEAT // 128        # 4 feature tiles (= head pairs)
QT_N = S // 128           # 4 query tiles per batch element
NL_N = NLOC // 512        # 4 output column chunks
EXP_SCALE = float(2.0 ** -18)

_CACHE = {}


def _build(use_bias):
    nc = bacc.Bacc(
        "TRN2",
        target_bir_lowering=False,
        debug=False,
        enable_asserts=False,
        num_devices=NCORES,
    )

    # q/k pre-transposed+tiled fp8: [p, i, r] = x[r, i*128+p].
    d_q8 = nc.dram_tensor("q8", [128, KT_N, R], FP8, kind="ExternalInput").ap()
    d_k8 = nc.dram_tensor("k8", [128, KT_N, R], FP8, kind="ExternalInput").ap()
    d_v = nc.dram_tensor("v16", [128, KT_N, R], BF16, kind="ExternalInput").ap()
    # masked str pre-tiled bf16: [p, b, qt*S+c] (mask folded in as -1e4).
    d_str = nc.dram_tensor("strm", [128, BL, QT_N * S], BF16, kind="ExternalInput").ap()
    # weights pre-tiled: [p, i, f] = W.T[i*128+p, f] (wq/wk scaled by 64, fp8).
    d_wq8 = nc.dram_tensor("wq8", [128, KT_N, FEAT], FP8, kind="ExternalInput").ap()
    d_wk8 = nc.dram_tensor("wk8", [128, KT_N, FEAT], FP8, kind="ExternalInput").ap()
    d_wv = nc.dram_tensor("wv16", [128, KT_N, FEAT], BF16, kind="ExternalInput").ap()
    d_wo = nc.dram_tensor("wo16", [128, FT_N, NLOC], BF16, kind="ExternalInput").ap()
    d_id = nc.dram_tensor("ident", [128, 128], BF16, kind="ExternalInput").ap()
    # biases (only used when nonzero): bq/bk pre-scaled by 64, feat-major.
    d_bq = nc.dram_tensor("bq2", [128, FT_N], F32, kind="ExternalInput").ap()
    d_bk = nc.dram_tensor("bk2", [128, FT_N], F32, kind="ExternalInput").ap()
    d_bv = nc.dram_tensor("bv2", [1, FEAT], BF16, kind="ExternalInput").ap()
    d_bo = nc.dram_tensor("bo2", [1, NLOC], BF16, kind="ExternalInput").ap()
    d_out = nc.dram_tensor("out", [R, NLOC], BF16, kind="ExternalOutput").ap()

    with tile.TileContext(nc) as tc:
        with (
            tc.tile_pool(name="consts", bufs=1) as cpool,
            tc.tile_pool(name="weights", bufs=1) as wpool,
            tc.tile_pool(name="persist", bufs=1) as ppool,
            tc.tile_pool(name="qkin", bufs=8) as qkpool,
            tc.tile_pool(name="vin", bufs=1) as vpool,
            tc.tile_pool(name="smwork", bufs=1) as mpool,
            tc.tile_pool(name="diagp", bufs=4) as dgpool,
            tc.tile_pool(name="attn", bufs=4) as apool,
            tc.tile_pool(name="recs", bufs=2) as recpool,
            tc.tile_pool(name="rts", bufs=4) as rtpool,
            tc.tile_pool(name="xoddp", bufs=2) as xopool,
            tc.tile_pool(name="ostage", bufs=2) as opool,
            tc.tile_pool(name="psumA", bufs=2, space="PSUM") as psA,
            tc.tile_pool(name="psumB", bufs=2, space="PSUM") as psB,
            tc.tile_pool(name="psumP", bufs=2, space="PSUM") as psP,
        ):
            # ---- constants ----------------------------------------------
            ident = cpool.tile([128, 128], BF16, tag="ident", name="ident")
            ones = cpool.tile([128, 128], BF16, tag="ones", name="ones")
            nc.vector.memset(ones[:], 1.0)
            nc.scalar.dma_start(ident[:], d_id[:])

            # str (mask pre-folded) on the ACT hwdge ring.
            strb = [None, None]
            for b in range(BL):
                strb[b] = mpool.tile([128, QT_N, S], BF16, tag=f"str{b}", name=f"str{b}")
                nc.scalar.dma_start(
                    strb[b][:].rearrange("p a c -> p (a c)"), d_str[:, b, :]
                )

            biases = {}
            if use_bias:
                bq_t = cpool.tile([128, FT_N], F32, tag="bq", name="bq")
                nc.scalar.dma_start(bq_t[:], d_bq[:])
                bk_t = cpool.tile([128, FT_N], F32, tag="bk", name="bk")
                nc.scalar.dma_start(bk_t[:], d_bk[:])
                bv_t = cpool.tile([1, FEAT], BF16, tag="bv", name="bv")
                nc.scalar.dma_start(bv_t[:], d_bv[:])
                bo_t = cpool.tile([1, NLOC], BF16, tag="bo", name="bo")
                nc.scalar.dma_start(bo_t[:], d_bo[:])
                biases = {"bq": bq_t, "bk": bk_t, "bv": bv_t, "bo": bo_t}

            # ---- structural softmax prep (ACT/DVE, overlaps Q proj) ------
            # u[b] = exp(masked str) with fused rowsums; diag = ident*ru
            u_t = [None, None]
            diag = [[None] * QT_N, [None] * QT_N]
            for b in range(BL):
                u_t[b] = mpool.tile([128, QT_N, S], BF16, tag=f"u{b}", name=f"u{b}")
                usum = mpool.tile([128, QT_N], F32, tag=f"us{b}", name=f"us{b}")
                for qt in range(QT_N):
                    nc.scalar.activation(
                        u_t[b][:, qt, :], strb[b][:, qt, :], AF.Exp,
                        accum_out=usum[:, qt : qt + 1],
                    )
                ru = mpool.tile([128, QT_N], F32, tag=f"ru{b}", name=f"ru{b}")
                nc.vector.reciprocal_approx_fast(ru[:], usum[:])
                ru2 = mpool.tile([128, QT_N], F32, tag=f"ru2{b}", name=f"ru2{b}")
                nc.vector.tensor_scalar(
                    ru2[:], ru[:], float(2.0 ** 18), None, op0=ALU.mult
                )
                for qt in range(QT_N):
                    dg = dgpool.tile([128, 128], BF16, tag=f"dg{b}{qt}", name=f"dg{b}{qt}")
                    nc.vector.tensor_scalar(
                        dg[:], ident[:], ru2[:, qt : qt + 1], None, op0=ALU.mult
                    )
                    diag[b][qt] = dg

            # ---- persistent activations ---------------------------------
            QT = [ppool.tile([128, R], BF16, tag=f"QT{i}", name=f"QT{i}") for i in range(FT_N)]
            KT = [ppool.tile([128, R], BF16, tag=f"KT{i}", name=f"KT{i}") for i in range(FT_N)]
            V2 = [ppool.tile([128, H, DH + 1], BF16, tag=f"V2{i}", name=f"V2{i}") for i in range(R // 128)]
            # smTs[b][kt]: 2^18 * sm^T, bf16
            smTs = [[None] * QT_N, [None] * QT_N]
            xT = [
                [ppool.tile([128, S], BF16, tag=f"xT{b}{j}", name=f"xT{b}{j}") for j in range(FT_N)]
                for b in range(BL)
            ]

            def grab_half(idx):
                # 8 concurrent [128,512] projection groups: 2x2 halves from
                # psA ([128,1024] tiles) + 2 from psB + 2 from psP.
                if idx < 4:
                    if idx % 2 == 0:
                        t = psA.tile([128, 1024], F32, tag="ps", name="ps")
                        grab_half._last = t
                    t = grab_half._last
                    return t[:, (idx % 2) * 512 : (idx % 2 + 1) * 512]
                pool = psB if idx < 6 else psP
                return pool.tile([128, 512], F32, tag="ps", name="ps")[:]

            def proj_fp8(dst, d_x, d_w, w_tag, bias_nm):
                w_t = wpool.tile([128, KT_N, FEAT], FP8, tag=w_tag, name=w_tag)
                groups = {}
                for ft in range(FT_N):
                    for rc in range(2):
                        groups[(ft, rc)] = grab_half(ft * 2 + rc)
                for i in range(KT_N // 2):
                    if i % 2 == 0:
                        j = i // 2
                        nc.sync.dma_start(
                            w_t[:, 4 * j : 4 * j + 4, :], d_w[:, 4 * j : 4 * j + 4, :]
                        )
                    xc = qkpool.tile([128, 2, R], FP8, tag="xin", name="xin")
                    nc.sync.dma_start(xc[:], d_x[:, 2 * i : 2 * i + 2, :])
                    for ft in range(FT_N):
                        for rc in range(2):
                            nc.tensor.matmul(
                                groups[(ft, rc)],
                                lhsT=w_t[:, 2 * i : 2 * i + 2, ft * 128 : (ft + 1) * 128],
                                rhs=xc[:, :, rc * 512 : (rc + 1) * 512],
                                start=(i == 0),
                                stop=(i == KT_N // 2 - 1),
                                perf_mode=DR,
                            )
                for ft in range(FT_N):
                    for rc in range(2):
                        if use_bias:
                            nc.vector.tensor_scalar(
                                dst[ft][:, rc * 512 : (rc + 1) * 512],
                                groups[(ft, rc)],
                                biases[bias_nm][:, ft : ft + 1],
                                None,
                                op0=ALU.add,
                            )
                        else:
                            nc.vector.tensor_copy(
                                dst[ft][:, rc * 512 : (rc + 1) * 512],
                                groups[(ft, rc)],
                            )

            def smt_phase(b):
                # sm^T via diag-scaled PE transpose, exp on ACT -> esmT bf16
                for kt in range(QT_N):
                    smtp = psB.tile([128, 512], F32, tag="ps", name="smtp")
                    for qt in range(QT_N):
                        nc.tensor.matmul(
                            smtp[:, qt * 128 : (qt + 1) * 128],
                            lhsT=u_t[b][:, qt, kt * 128 : (kt + 1) * 128],
                            rhs=diag[b][qt][:],
                            start=(qt == 0),
                            stop=(qt == QT_N - 1),
                        )
                    sts = ppool.tile([128, S], BF16, tag=f"sts{b}{kt}", name=f"sts{b}{kt}")
                    nc.vector.tensor_copy(sts[:], smtp[:])
                    smTs[b][kt] = sts

            proj_fp8(QT, d_q8, d_wq8, "wq8", "bq")
            smt_phase(0)
            proj_fp8(KT, d_k8, d_wk8, "wk8", "bk")
            smt_phase(1)

            # ---- V projection (bf16, token-major, ones-augmented) --------
            # Group-major (per row-tile) over fully-staged v halves, using
            # only psB banks so batch-0 attention can interleave on psA.
            wv = wpool.tile([128, KT_N, FEAT], BF16, tag="wv", name="wv")
            nc.sync.dma_start(wv[:], d_wv[:])
            vst = []
            for half in range(2):
                vh = vpool.tile([128, KT_N, 512], BF16, tag=f"vst{half}", name=f"vst{half}")
                nc.sync.dma_start(vh[:], d_v[:, :, half * 512 : (half + 1) * 512])
                vst.append(vh)
            # wo after all other inputs on the SP ring.
            wo = wpool.tile([128, FT_N, NLOC], BF16, tag="wo", name="wo")
            nc.sync.dma_start(wo[:], d_wo[:])

            def v_group(rt):
                vg = psP.tile([128, 512], F32, tag="ps", name="vg")
                if use_bias:
                    nc.tensor.matmul(
                        vg[:], lhsT=ones[0:1, :], rhs=biases["bv"][0:1, :],
                        start=True, stop=False,
                    )
                vh = vst[rt // 4]
                for i in range(KT_N):
                    nc.tensor.matmul(
                        vg[:],
                        lhsT=vh[:, i, (rt % 4) * 128 : (rt % 4 + 1) * 128],
                        rhs=wv[:, i, :],
                        start=(i == 0 and not use_bias),
                        stop=(i == KT_N - 1),
                    )
                nc.vector.memset(V2[rt][:], 1.0)
                nc.vector.tensor_copy(
                    V2[rt][:, :, 0:DH],
                    vg[:].rearrange("p (h d) -> p h d", h=H),
                )

            # ---- attention (transposed scores) ---------------------------
            ets = {}

            def scores_part(b, hp):
                for kt in range(QT_N):
                    sp = psA.tile([128, 1024], F32, tag="ps", name="ps")
                    for hs in range(2):
                        hb = hs * 64
                        nc.tensor.matmul(
                            sp[:, hs * 512 : (hs + 1) * 512],
                            lhsT=KT[hp][hb : hb + 64, b * S + kt * 128 : b * S + (kt + 1) * 128],
                            rhs=QT[hp][hb : hb + 64, b * S : (b + 1) * S],
                            start=True,
                            stop=False,
                        )
                    for hs in range(2):
                        nc.tensor.matmul(
                            sp[:, hs * 512 : (hs + 1) * 512],
                            lhsT=ident[:],
                            rhs=smTs[b][kt][:],
                            start=False,
                            stop=True,
                        )
                    et = apool.tile([128, 1024], BF16, tag="et", name="et")
                    for hs in range(2):
                        nc.scalar.activation(
                            et[:, hs * 512 : (hs + 1) * 512],
                            sp[:, hs * 512 : (hs + 1) * 512],
                            AF.Exp,
                            scale=EXP_SCALE,
                        )
                    ets[(b, hp, kt)] = et

            def pv_part(b, hp):
                yps = [psB.tile([128, 512], F32, tag="ps", name="yps") for _ in range(2)]
                for kt in range(QT_N):
                    for hs in range(2):
                        h = 2 * hp + hs
                        nc.tensor.matmul(
                            yps[hs][0:65, :],
                            lhsT=V2[b * QT_N + kt][:, h, :],
                            rhs=ets[(b, hp, kt)][:, hs * 512 : (hs + 1) * 512],
                            start=(kt == 0),
                            stop=(kt == QT_N - 1),
                        )
                    ets[(b, hp, kt)] = None
                # normalization: copy raw rowsums to SBUF (ACT), broadcast
                # with a K=1 PE matmul, approx-reciprocal, multiply to xT.
                recp = recpool.tile([65, 1024], BF16, tag="rec", name="recp")
                nc.scalar.activation(recp[64:65, 0:512], yps[0][64:65, :], AF.Copy)
                nc.scalar.activation(recp[64:65, 512:1024], yps[1][64:65, :], AF.Copy)
                rtp = psA.tile([128, 1024], F32, tag="ps", name="rtp")
                for hs in range(2):
                    nc.tensor.matmul(
                        rtp[0:64, hs * 512 : (hs + 1) * 512],
                        lhsT=ones[64:65, 0:64],
                        rhs=recp[64:65, hs * 512 : (hs + 1) * 512],
                        start=True,
                        stop=True,
                    )
                rsb = rtpool.tile([64, 1024], F32, tag="rt", name="rt")
                nc.vector.reciprocal_approx_fast(rsb[:], rtp[0:64, :])
                nc.vector.tensor_tensor(
                    xT[b][hp][0:64, :], yps[0][0:64, :], rsb[:, 0:512], op=ALU.mult
                )
                xod = xopool.tile([64, 512], BF16, tag="xod", name="xod")
                nc.vector.tensor_tensor(
                    xod[:], yps[1][0:64, :], rsb[:, 512:1024], op=ALU.mult
                )
                nc.gpsimd.dma_start(xT[b][hp][64:128, :], xod[:])

            def outproj_qt(b, qt, copy_engine):
                row0 = b * S + qt * 128
                ot = opool.tile([128, NLOC], BF16, tag="ot", name="ot")
                for nlc in range(NL_N):
                    ops = psP.tile([128, 512], F32, tag="ps", name="ops")
                    if use_bias:
                        nc.tensor.matmul(
                            ops[:],
                            lhsT=ones[0:1, :],
                            rhs=biases["bo"][0:1, nlc * 512 : (nlc + 1) * 512],
                            start=True,
                            stop=False,
                        )
                    for ft in range(FT_N):
                        nc.tensor.matmul(
                            ops[:],
                            lhsT=xT[b][ft][:, qt * 128 : (qt + 1) * 128],
                            rhs=wo[:, ft, nlc * 512 : (nlc + 1) * 512],
                            start=(ft == 0 and not use_bias),
                            stop=(ft == FT_N - 1),
                        )
                    use_act = copy_engine == "act" or (
                        copy_engine == "mix" and nlc % 2 == 0
                    )
                    if use_act:
                        nc.scalar.activation(
                            ot[:, nlc * 512 : (nlc + 1) * 512], ops[:], AF.Copy
                        )
                    else:
                        nc.vector.tensor_copy(
                            ot[:, nlc * 512 : (nlc + 1) * 512], ops[:]
                        )
                    if nlc % 2 == 1:
                        ring = nc.gpsimd if (b == 1 and nlc == 1) else nc.sync
                        ring.dma_start(
                            d_out[row0 : row0 + 128, (nlc - 1) * 512 : (nlc + 1) * 512],
                            ot[:, (nlc - 1) * 512 : (nlc + 1) * 512],
                        )

            # batch-0 attention interleaved into the V projection groups
            for rt in range(4):
                v_group(rt)
            scores_part(0, 0)
            v_group(4)
            scores_part(0, 1)
            pv_part(0, 0)
            v_group(5)
            scores_part(0, 2)
            pv_part(0, 1)
            v_group(6)
            scores_part(0, 3)
            pv_part(0, 2)
            v_group(7)
            pv_part(0, 3)
            # batch-1 attention with batch-0 out-proj interleaved
            for hp in range(FT_N):
                scores_part(1, hp)
                pv_part(1, hp)
                outproj_qt(0, hp, "dve")
            for qt in range(QT_N):
                outproj_qt(1, qt, "mix")

    nc.compile()
    return nc


def _prep_inputs(q, k, v, str_mat, attn_mask, Wq, bq, Wk, bk, Wv, bv, Wo, bo):
    bf = ml_dtypes.bfloat16
    f8 = ml_dtypes.float8_e4m3

    def pretile_T(x, dt):
        # [R, NLOC] -> [128, KT_N, R] with [p, i, r] = x[r, i*128+p]
        return np.ascontiguousarray(
            x.reshape(R, KT_N, 128).transpose(2, 1, 0).astype(dt)
        )

    def pretile_w(wT, n, dt):
        # [n*128, width] -> [128, n, width]
        return np.ascontiguousarray(
            wT.reshape(n, 128, wT.shape[1]).transpose(1, 0, 2).astype(dt)
        )

    wq8 = pretile_w((64.0 * Wq).T.astype(np.float32), KT_N, f8)
    wk8 = pretile_w((64.0 * Wk).T.astype(np.float32), KT_N, f8)
    wv16 = pretile_w(np.ascontiguousarray(Wv.T), KT_N, bf)
    wo16 = pretile_w(np.ascontiguousarray(Wo.T), FT_N, bf)

    ident = np.eye(128, dtype=bf)
    bq2 = np.ascontiguousarray((64.0 * bq).astype(np.float32).reshape(FT_N, 128).T)
    bk2 = np.ascontiguousarray((64.0 * bk).astype(np.float32).reshape(FT_N, 128).T)
    bv2 = bv[None, :].astype(bf)
    bo2 = bo[None, :].astype(bf)

    # fold attn_mask into str: masked entries -> -1e4 (exp -> 0 in f32)
    strf = np.where(
        np.asarray(attn_mask) == 0, np.float32(-1e4),
        np.asarray(str_mat, dtype=np.float32),
    )

    in_maps = []
    for c in range(NCORES):
        sl = slice(c * BL, (c + 1) * BL)
        strt = np.ascontiguousarray(
            strf[sl].reshape(BL * QT_N, 128, S).transpose(1, 0, 2)
            .reshape(128, BL, QT_N * S).astype(bf)
        )
        in_maps.append(
            {
                "q8": pretile_T(np.asarray(q)[sl].reshape(R, NLOC), f8),
                "k8": pretile_T(np.asarray(k)[sl].reshape(R, NLOC), f8),
                "v16": pretile_T(np.asarray(v)[sl].reshape(R, NLOC), bf),
                "strm": strt,
                "wq8": wq8,
                "wk8": wk8,
                "wv16": wv16,
                "wo16": wo16,
                "ident": ident,
                "bq2": bq2,
                "bk2": bk2,
                "bv2": bv2,
                "bo2": bo2,
            }
        )
    return in_maps


def kernel(q, k, v, str_mat, attn_mask, Wq, bq, Wk, bk, Wv, bv, Wo, bo):
    use_bias = bool(
        np.any(np.asarray(bq))
        or np.any(np.asarray(bk))
        or np.any(np.asarray(bv))
        or np.any(np.asarray(bo))
    )
    key = ("nc", use_bias)
    if key not in _CACHE:
        _CACHE[key] = _build(use_bias)
    nc = _CACHE[key]
    in_maps = _prep_inputs(
        q, k, v, str_mat, attn_mask, Wq, bq, Wk, bk, Wv, bv, Wo, bo
    )
    res = run_bass_kernel_spmd(nc, in_maps, core_ids=list(range(NCORES)))
    out = np.empty((B, S, NLOC), dtype=np.float32)
    for c in range(NCORES):
        out[c * BL : (c + 1) * BL] = (
            res.results[c]["out"].astype(np.float32).reshape(BL, S, NLOC)
        )
    return out
